# revision 18
# baseline (speedup 1.0000x reference)
"""Binarized 3x3 conv2d (hardtanh activation, clipped reweight ++ plain conv_w)
run data-parallel across 8 Trainium2 NeuronCores.

Math: out = conv2d(clip(x,-1,1), concat(clip(reweight,-1,1), conv_w)), pad=1
— a single 128->128 channel 3x3 conv (the two branches just split the output
channels), so the weights are fused + pre-transposed on the host and the conv
runs as one kernel.

Sharding: pure data parallel — batch 32 -> 4 images per core, weights
replicated (tiny). No collectives needed (forward only).

Per-core kernel: Cin=128 lives on SBUF partitions.  Each image is clipped and
cast to fp16 into a zero-padded [128, 114, 114] SBUF buffer (DMA + clip run
in 16 row-chunks so the first matmuls start early; all four clipped fp16
images stay SBUF-resident — a_pad bufs=4 — so image boundaries never stall
the PE).  Each group of R=4 output
rows is one PSUM bank accumulating 9 shifted full-size fp16 matmuls
(moving operand [128, 4, 112] strided slices of the padded image; stationary
operand the per-tap [Cin=128, Cout=128] weight slice):
  psum[co, r*112+w] += sum_ci W[kh,kw][ci,co] * a_pad[ci, h0+r+kh+1, w+kw+1]
PSUM blocks are copied to SBUF on the scalar engine and DMA'd out from there
(same engine => program-order WAR, which keeps every instruction within the
2-sync-command hardware limit).

fp16 (not bf16): same 1 cycle/row PE throughput, 3 more mantissa bits.
Inputs are in [-1,1] and weights ~0.05 so fp16 range is safe; accumulation is
fp32 in PSUM.  Measured absmax-relative error vs fp32 reference: 2.3e-4.
Measured steady-state HW time: ~184 us/core (PE roofline: 9 taps x 12544 px
x 4 images / 2.4 GHz = 188 us; DMA 51 MB / 358 GB/s = 143 us).
"""

import time as _time

import numpy as np
from contextlib import ExitStack

import concourse.bass as bass
import concourse.mybir as mybir
import concourse.tile as tile
from concourse import bacc

B, C, H, W = 32, 128, 112, 112
NCORES = 8
BPC = B // NCORES  # images per core
R = 4              # output rows per PSUM block (R*W = 448 <= 512 psum bank)

MODE = "fp16"  # "fp16" | "bf16" | "f32"

_nc_cache: dict = {}
_runner_cache: dict = {}


def _build(mode: str, bpc: int = BPC, h: int = H, w: int = W, reps: int = 1) -> bass.Bass:
    f32 = mybir.dt.float32
    wdt = {"bf16": mybir.dt.bfloat16, "fp16": mybir.dt.float16}.get(mode, f32)
    adt = wdt  # matmul moving-operand dtype as stored in SBUF
    hp, wp = h + 2, w + 2
    assert h % R == 0

    nc = bacc.Bacc("TRN2", target_bir_lowering=False, debug=False)
    x_in = nc.declare_dram_parameter("x", [bpc, C, h, w], f32, isOutput=False)
    w_in = nc.declare_dram_parameter("w", [C, 9, C], wdt, isOutput=False)
    out_d = nc.declare_dram_parameter("out", [bpc, C, h, w], f32, isOutput=True)

    taps = [(kh, kw) for kh in (-1, 0, 1) for kw in (-1, 0, 1)]

    with tile.TileContext(nc) as tc, ExitStack() as ctx:
        wpool = ctx.enter_context(tc.tile_pool(name="wpool", bufs=1))
        apool = ctx.enter_context(tc.tile_pool(name="apool", bufs=2))
        opool = ctx.enter_context(tc.tile_pool(name="opool", bufs=6))
        pspool = ctx.enter_context(tc.tile_pool(name="pspool", bufs=8, space="PSUM"))

        w_s = wpool.tile([C, 9, C], wdt)
        nc.sync.dma_start(w_s[:], w_in[:])

        for b_outer in range(bpc * reps):
            b = b_outer % bpc
            if mode in ("bf16", "fp16"):
                a_raw = apool.tile([C, h, w], f32, tag="a_raw", bufs=1)
                a = apool.tile([C, hp, wp], adt, tag="a_pad", bufs=4)
                # zero the 1-wide border, clip+cast the interior in row chunks
                # (chunking lets the first matmuls start before the whole
                # image has arrived / been clipped)
                nc.vector.memset(a[:, 0, :], 0.0)
                nc.vector.memset(a[:, hp - 1, :], 0.0)
                nc.vector.memset(a[:, 1 : hp - 1, 0], 0.0)
                nc.vector.memset(a[:, 1 : hp - 1, wp - 1], 0.0)
                nchunk = min(16, h)
                cr = h // nchunk
                for ci in range(nchunk):
                    r0, r1 = ci * cr, (ci + 1) * cr
                    nc.sync.dma_start(a_raw[:, r0:r1, :], x_in[b][:, r0:r1, :])
                    nc.vector.tensor_scalar(
                        out=a[:, 1 + r0 : 1 + r1, 1 : wp - 1], in0=a_raw[:, r0:r1, :],
                        scalar1=1.0, scalar2=-1.0,
                        op0=mybir.AluOpType.min, op1=mybir.AluOpType.max,
                    )
            else:
                a = apool.tile([C, hp, wp], f32, tag="a_pad")
                nc.vector.memset(a[:, 0, :], 0.0)
                nc.vector.memset(a[:, hp - 1, :], 0.0)
                nc.vector.memset(a[:, 1 : hp - 1, 0], 0.0)
                nc.vector.memset(a[:, 1 : hp - 1, wp - 1], 0.0)
                nc.sync.dma_start(a[:, 1 : hp - 1, 1 : wp - 1], x_in[b])
                # clip the whole padded buffer in place (clip(0)=0)
                nc.vector.tensor_scalar(
                    out=a[:], in0=a[:], scalar1=1.0, scalar2=-1.0,
                    op0=mybir.AluOpType.min, op1=mybir.AluOpType.max,
                )

            out_flat = out_d[b].rearrange("c h w -> c (h w)")
            for h0 in range(0, h, R):
                ps = pspool.tile([C, R * w], f32)
                for i, (kh, kw) in enumerate(taps):
                    # input rows h0+kh+1 .. +R, cols kw+1 .. +w (padded coords)
                    rhs = a[:, h0 + kh + 1 : h0 + kh + 1 + R, kw + 1 : kw + 1 + w]
                    lhsT = w_s[:, (kh + 1) * 3 + (kw + 1), :]
                    nc.tensor.matmul(
                        ps[:], lhsT, rhs,
                        start=(i == 0), stop=(i == len(taps) - 1),
                    )

                ot = opool.tile([C, R * w], f32)
                nc.scalar.copy(ot[:], ps[:])
                nc.scalar.dma_start(out_flat[:, h0 * w : (h0 + R) * w], ot[:])

    nc.compile()
    return nc


def _prep_weights(reweight: np.ndarray, conv_w: np.ndarray, mode: str) -> np.ndarray:
    w_full = np.concatenate([np.clip(reweight, -1.0, 1.0), conv_w], axis=0)  # [128,128,3,3]
    w_k = np.ascontiguousarray(w_full.transpose(1, 2, 3, 0)).reshape(C, 9, C)  # [ci,k,co]
    if mode == "bf16":
        import ml_dtypes
        w_k = w_k.astype(ml_dtypes.bfloat16)
    elif mode == "fp16":
        w_k = w_k.astype(np.float16)
    return w_k


def _get_nc(mode: str):
    if mode not in _nc_cache:
        _nc_cache[mode] = _build(mode)
    return _nc_cache[mode]


class _Runner:
    """Persistent jitted shard_map executor for one compiled Bass module.

    Mirrors concourse.bass2jax.run_bass_via_pjrt's multi-core path, but keeps
    the jitted function (and the donated output buffers) alive across calls so
    repeated kernel() invocations skip recompilation.  Output buffers are
    donation-chained: the kernel writes every output element, so reusing the
    previous call's outputs as the next call's output buffers is safe.
    """

    def __init__(self, nc, n_cores: int):
        import jax
        from concourse import bass2jax
        from jax.experimental.shard_map import shard_map
        from jax.sharding import Mesh, NamedSharding, PartitionSpec

        bass2jax.install_neuronx_cc_hook()
        self.jax = jax
        self.n_cores = n_cores
        partition_name = nc.partition_id_tensor.name if nc.partition_id_tensor else None
        in_names, out_names, out_avals = [], [], []
        for alloc in nc.m.functions[0].allocations:
            if not isinstance(alloc, mybir.MemoryLocationSet):
                continue
            name = alloc.memorylocations[0].name
            if alloc.kind == "ExternalInput":
                if name != partition_name:
                    in_names.append(name)
            elif alloc.kind == "ExternalOutput":
                out_names.append(name)
                out_avals.append(
                    jax.core.ShapedArray(
                        tuple(alloc.tensor_shape), mybir.dt.np(alloc.dtype)
                    )
                )
        self.in_names, self.out_names, self.out_avals = in_names, out_names, out_avals
        n_params = len(in_names)
        all_in_names = list(in_names) + list(out_names)
        if partition_name is not None:
            all_in_names.append(partition_name)
        donate = tuple(range(n_params, n_params + len(out_names)))

        def _body(*args):
            operands = list(args)
            if partition_name is not None:
                operands.append(bass2jax.partition_id_tensor())
            return tuple(
                bass2jax._bass_exec_p.bind(
                    *operands,
                    out_avals=tuple(out_avals),
                    in_names=tuple(all_in_names),
                    out_names=tuple(out_names),
                    lowering_input_output_aliases=(),
                    sim_require_finite=True,
                    sim_require_nnan=True,
                    nc=nc,
                )
            )

        devices = jax.devices()[:n_cores]
        assert len(devices) >= n_cores, f"need {n_cores} devices, got {len(devices)}"
        mesh = Mesh(np.asarray(devices), ("core",))
        self.sharding = NamedSharding(mesh, PartitionSpec("core"))
        self.sharded = jax.jit(
            shard_map(
                _body, mesh=mesh,
                in_specs=(PartitionSpec("core"),) * (n_params + len(out_names)),
                out_specs=(PartitionSpec("core"),) * len(out_names),
                check_rep=False,
            ),
            donate_argnums=donate, keep_unused=True,
        )
        self._outs = None  # donated output buffers, chained across calls

    def __call__(self, in_maps):
        jax = self.jax
        per_core = [[np.asarray(m[name]) for name in self.in_names] for m in in_maps]
        concat_in = [
            np.concatenate([per_core[c][i] for c in range(self.n_cores)], axis=0)
            for i in range(len(self.in_names))
        ]
        xin = [jax.device_put(a, self.sharding) for a in concat_in]
        if self._outs is None:
            self._outs = [
                jax.device_put(
                    np.zeros((self.n_cores * av.shape[0], *av.shape[1:]), av.dtype),
                    self.sharding,
                )
                for av in self.out_avals
            ]
        self._outs = list(self.sharded(*xin, *self._outs))
        out_np = [np.asarray(o) for o in self._outs]
        return [
            {
                name: out_np[i].reshape(self.n_cores, *self.out_avals[i].shape)[c]
                for i, name in enumerate(self.out_names)
            }
            for c in range(self.n_cores)
        ]


def _run_spmd(nc, in_maps, mode: str):
    last = None
    for attempt in range(3):
        try:
            if mode not in _runner_cache:
                _runner_cache[mode] = _Runner(nc, NCORES)
            return _runner_cache[mode](in_maps)
        except Exception as e:
            last = e
            _runner_cache.pop(mode, None)
        # fall back to the stock one-shot path (also covers transient
        # device/terminal wedges, with a pause between attempts)
        try:
            from concourse.bass_utils import run_bass_kernel_spmd

            return run_bass_kernel_spmd(nc, in_maps, list(range(NCORES))).results
        except Exception as e:
            last = e
            _time.sleep(15)
    raise last


def run(x, reweight, conv_w, mode: str | None = None):
    mode = mode or MODE
    nc = _get_nc(mode)
    w_k = _prep_weights(np.asarray(reweight), np.asarray(conv_w), mode)
    x = np.ascontiguousarray(np.asarray(x), dtype=np.float32)
    in_maps = [
        {"x": np.ascontiguousarray(x[i * BPC : (i + 1) * BPC]), "w": w_k}
        for i in range(NCORES)
    ]
    results = _run_spmd(nc, in_maps, mode)
    return np.concatenate([results[i]["out"] for i in range(NCORES)], axis=0)


def kernel(x, reweight, conv_w):
    return run(x, reweight, conv_w)


# revision 20
# speedup vs baseline: 1.0622x; 1.0622x over previous
"""Binarized 3x3 conv2d (hardtanh activation, clipped reweight ++ plain conv_w)
run data-parallel across 8 Trainium2 NeuronCores.

Math: out = conv2d(clip(x,-1,1), concat(clip(reweight,-1,1), conv_w)), pad=1
— a single 128->128 channel 3x3 conv (the two branches just split the output
channels), so the weights are fused + pre-transposed on the host and the conv
runs as one kernel.

Sharding: pure data parallel — batch 32 -> 4 images per core, weights
replicated (tiny). No collectives needed (forward only).

Per-core kernel: Cin=128 lives on SBUF partitions.  Each image is clipped and
cast to fp16 into a zero-padded [128, 114, 114] SBUF buffer.  DMA + clip run
in row chunks (image 0 leads with 2/2/4/8-row chunks so the first matmuls
start ~2 us in; the rest use 16-row chunks), and all four clipped fp16 images
stay SBUF-resident (a_pad bufs=4) so image boundaries never stall the PE.
Each group of R=4 output rows is one PSUM bank accumulating 9 shifted
full-size fp16 matmuls
(moving operand [128, 4, 112] strided slices of the padded image; stationary
operand the per-tap [Cin=128, Cout=128] weight slice):
  psum[co, r*112+w] += sum_ci W[kh,kw][ci,co] * a_pad[ci, h0+r+kh+1, w+kw+1]
PSUM blocks are copied to SBUF on the scalar engine and DMA'd out from there
(same engine => program-order WAR, which keeps every instruction within the
2-sync-command hardware limit).

fp16 (not bf16): same 1 cycle/row PE throughput, 3 more mantissa bits.
Inputs are in [-1,1] and weights ~0.05 so fp16 range is safe; accumulation is
fp32 in PSUM.  Measured absmax-relative error vs fp32 reference: 2.3e-4.
Measured steady-state HW time: 184-194 us/core across sessions (PE roofline:
9 taps x 12544 px x 4 images / 2.4 GHz = 188 us; DMA 51 MB / 358 GB/s =
143 us); cost-model one-shot estimate 201 us.
"""

import time as _time

import numpy as np
from contextlib import ExitStack

import concourse.bass as bass
import concourse.mybir as mybir
import concourse.tile as tile
from concourse import bacc

B, C, H, W = 32, 128, 112, 112
NCORES = 8
BPC = B // NCORES  # images per core
R = 4              # output rows per PSUM block (R*W = 448 <= 512 psum bank)

MODE = "fp16"  # "fp16" | "bf16" | "f32"

_nc_cache: dict = {}
_runner_cache: dict = {}


def _build(mode: str, bpc: int = BPC, h: int = H, w: int = W, reps: int = 1) -> bass.Bass:
    f32 = mybir.dt.float32
    wdt = {"bf16": mybir.dt.bfloat16, "fp16": mybir.dt.float16}.get(mode, f32)
    adt = wdt  # matmul moving-operand dtype as stored in SBUF
    hp, wp = h + 2, w + 2
    assert h % R == 0

    nc = bacc.Bacc("TRN2", target_bir_lowering=False, debug=False)
    x_in = nc.declare_dram_parameter("x", [bpc, C, h, w], f32, isOutput=False)
    w_in = nc.declare_dram_parameter("w", [C, 9, C], wdt, isOutput=False)
    out_d = nc.declare_dram_parameter("out", [bpc, C, h, w], f32, isOutput=True)

    taps = [(kh, kw) for kh in (-1, 0, 1) for kw in (-1, 0, 1)]

    with tile.TileContext(nc) as tc, ExitStack() as ctx:
        wpool = ctx.enter_context(tc.tile_pool(name="wpool", bufs=1))
        apool = ctx.enter_context(tc.tile_pool(name="apool", bufs=2))
        opool = ctx.enter_context(tc.tile_pool(name="opool", bufs=6))
        pspool = ctx.enter_context(tc.tile_pool(name="pspool", bufs=8, space="PSUM"))

        w_s = wpool.tile([C, 9, C], wdt)
        nc.sync.dma_start(w_s[:], w_in[:])

        for b_outer in range(bpc * reps):
            b = b_outer % bpc
            if mode in ("bf16", "fp16"):
                a_raw = apool.tile([C, h, w], f32, tag="a_raw", bufs=1)
                a = apool.tile([C, hp, wp], adt, tag="a_pad", bufs=4)
                # zero the 1-wide border, clip+cast the interior in row chunks
                # (chunking lets the first matmuls start before the whole
                # image has arrived / been clipped)
                nc.vector.memset(a[:, 0, :], 0.0)
                nc.vector.memset(a[:, hp - 1, :], 0.0)
                nc.vector.memset(a[:, 1 : hp - 1, 0], 0.0)
                nc.vector.memset(a[:, 1 : hp - 1, wp - 1], 0.0)
                # image 0 leads with tiny chunks so the first matmuls can
                # start ~2us earlier; later images are fully overlapped
                if h >= 32 and b_outer == 0:
                    sched = [2, 2, 4, 8] + [16] * ((h - 16) // 16)
                elif h >= 32:
                    sched = [16] * (h // 16)
                else:
                    sched = [h]
                bounds = [0]
                for sz in sched:
                    bounds.append(bounds[-1] + sz)
                for ci in range(len(sched)):
                    r0, r1 = bounds[ci], bounds[ci + 1]
                    nc.sync.dma_start(a_raw[:, r0:r1, :], x_in[b][:, r0:r1, :])
                    nc.vector.tensor_scalar(
                        out=a[:, 1 + r0 : 1 + r1, 1 : wp - 1], in0=a_raw[:, r0:r1, :],
                        scalar1=1.0, scalar2=-1.0,
                        op0=mybir.AluOpType.min, op1=mybir.AluOpType.max,
                    )
            else:
                a = apool.tile([C, hp, wp], f32, tag="a_pad")
                nc.vector.memset(a[:, 0, :], 0.0)
                nc.vector.memset(a[:, hp - 1, :], 0.0)
                nc.vector.memset(a[:, 1 : hp - 1, 0], 0.0)
                nc.vector.memset(a[:, 1 : hp - 1, wp - 1], 0.0)
                nc.sync.dma_start(a[:, 1 : hp - 1, 1 : wp - 1], x_in[b])
                # clip the whole padded buffer in place (clip(0)=0)
                nc.vector.tensor_scalar(
                    out=a[:], in0=a[:], scalar1=1.0, scalar2=-1.0,
                    op0=mybir.AluOpType.min, op1=mybir.AluOpType.max,
                )

            out_flat = out_d[b].rearrange("c h w -> c (h w)")
            for h0 in range(0, h, R):
                ps = pspool.tile([C, R * w], f32)
                for i, (kh, kw) in enumerate(taps):
                    # input rows h0+kh+1 .. +R, cols kw+1 .. +w (padded coords)
                    rhs = a[:, h0 + kh + 1 : h0 + kh + 1 + R, kw + 1 : kw + 1 + w]
                    lhsT = w_s[:, (kh + 1) * 3 + (kw + 1), :]
                    nc.tensor.matmul(
                        ps[:], lhsT, rhs,
                        start=(i == 0), stop=(i == len(taps) - 1),
                    )

                ot = opool.tile([C, R * w], f32)
                nc.scalar.copy(ot[:], ps[:])
                nc.scalar.dma_start(out_flat[:, h0 * w : (h0 + R) * w], ot[:])

    nc.compile()
    return nc


def _prep_weights(reweight: np.ndarray, conv_w: np.ndarray, mode: str) -> np.ndarray:
    w_full = np.concatenate([np.clip(reweight, -1.0, 1.0), conv_w], axis=0)  # [128,128,3,3]
    w_k = np.ascontiguousarray(w_full.transpose(1, 2, 3, 0)).reshape(C, 9, C)  # [ci,k,co]
    if mode == "bf16":
        import ml_dtypes
        w_k = w_k.astype(ml_dtypes.bfloat16)
    elif mode == "fp16":
        w_k = w_k.astype(np.float16)
    return w_k


def _get_nc(mode: str):
    if mode not in _nc_cache:
        _nc_cache[mode] = _build(mode)
    return _nc_cache[mode]


class _Runner:
    """Persistent jitted shard_map executor for one compiled Bass module.

    Mirrors concourse.bass2jax.run_bass_via_pjrt's multi-core path, but keeps
    the jitted function (and the donated output buffers) alive across calls so
    repeated kernel() invocations skip recompilation.  Output buffers are
    donation-chained: the kernel writes every output element, so reusing the
    previous call's outputs as the next call's output buffers is safe.
    """

    def __init__(self, nc, n_cores: int):
        import jax
        from concourse import bass2jax
        from jax.experimental.shard_map import shard_map
        from jax.sharding import Mesh, NamedSharding, PartitionSpec

        bass2jax.install_neuronx_cc_hook()
        self.jax = jax
        self.n_cores = n_cores
        partition_name = nc.partition_id_tensor.name if nc.partition_id_tensor else None
        in_names, out_names, out_avals = [], [], []
        for alloc in nc.m.functions[0].allocations:
            if not isinstance(alloc, mybir.MemoryLocationSet):
                continue
            name = alloc.memorylocations[0].name
            if alloc.kind == "ExternalInput":
                if name != partition_name:
                    in_names.append(name)
            elif alloc.kind == "ExternalOutput":
                out_names.append(name)
                out_avals.append(
                    jax.core.ShapedArray(
                        tuple(alloc.tensor_shape), mybir.dt.np(alloc.dtype)
                    )
                )
        self.in_names, self.out_names, self.out_avals = in_names, out_names, out_avals
        n_params = len(in_names)
        all_in_names = list(in_names) + list(out_names)
        if partition_name is not None:
            all_in_names.append(partition_name)
        donate = tuple(range(n_params, n_params + len(out_names)))

        def _body(*args):
            operands = list(args)
            if partition_name is not None:
                operands.append(bass2jax.partition_id_tensor())
            return tuple(
                bass2jax._bass_exec_p.bind(
                    *operands,
                    out_avals=tuple(out_avals),
                    in_names=tuple(all_in_names),
                    out_names=tuple(out_names),
                    lowering_input_output_aliases=(),
                    sim_require_finite=True,
                    sim_require_nnan=True,
                    nc=nc,
                )
            )

        devices = jax.devices()[:n_cores]
        assert len(devices) >= n_cores, f"need {n_cores} devices, got {len(devices)}"
        mesh = Mesh(np.asarray(devices), ("core",))
        self.sharding = NamedSharding(mesh, PartitionSpec("core"))
        self.sharded = jax.jit(
            shard_map(
                _body, mesh=mesh,
                in_specs=(PartitionSpec("core"),) * (n_params + len(out_names)),
                out_specs=(PartitionSpec("core"),) * len(out_names),
                check_rep=False,
            ),
            donate_argnums=donate, keep_unused=True,
        )
        self._outs = None  # donated output buffers, chained across calls

    def __call__(self, in_maps):
        jax = self.jax
        per_core = [[np.asarray(m[name]) for name in self.in_names] for m in in_maps]
        concat_in = [
            np.concatenate([per_core[c][i] for c in range(self.n_cores)], axis=0)
            for i in range(len(self.in_names))
        ]
        xin = [jax.device_put(a, self.sharding) for a in concat_in]
        if self._outs is None:
            self._outs = [
                jax.device_put(
                    np.zeros((self.n_cores * av.shape[0], *av.shape[1:]), av.dtype),
                    self.sharding,
                )
                for av in self.out_avals
            ]
        self._outs = list(self.sharded(*xin, *self._outs))
        out_np = [np.asarray(o) for o in self._outs]
        return [
            {
                name: out_np[i].reshape(self.n_cores, *self.out_avals[i].shape)[c]
                for i, name in enumerate(self.out_names)
            }
            for c in range(self.n_cores)
        ]


def _run_spmd(nc, in_maps, mode: str):
    last = None
    for attempt in range(3):
        try:
            if mode not in _runner_cache:
                _runner_cache[mode] = _Runner(nc, NCORES)
            return _runner_cache[mode](in_maps)
        except Exception as e:
            last = e
            _runner_cache.pop(mode, None)
        # fall back to the stock one-shot path (also covers transient
        # device/terminal wedges, with a pause between attempts)
        try:
            from concourse.bass_utils import run_bass_kernel_spmd

            return run_bass_kernel_spmd(nc, in_maps, list(range(NCORES))).results
        except Exception as e:
            last = e
            _time.sleep(15)
    raise last


def run(x, reweight, conv_w, mode: str | None = None):
    mode = mode or MODE
    nc = _get_nc(mode)
    w_k = _prep_weights(np.asarray(reweight), np.asarray(conv_w), mode)
    x = np.ascontiguousarray(np.asarray(x), dtype=np.float32)
    in_maps = [
        {"x": np.ascontiguousarray(x[i * BPC : (i + 1) * BPC]), "w": w_k}
        for i in range(NCORES)
    ]
    results = _run_spmd(nc, in_maps, mode)
    return np.concatenate([results[i]["out"] for i in range(NCORES)], axis=0)


def kernel(x, reweight, conv_w):
    return run(x, reweight, conv_w)
